# revision 52
# baseline (speedup 1.0000x reference)
"""Trainium2 Bass kernel for nn_HT_56298431316042 (histogram_binning).

Computes  out = relu(image.reshape(32, 16384)) @ vote.reshape(16384, 16384) / 128
         -> reshape (2, 16, 128, 128)

Sharding: column-wise over the 16384 Hough bins -> 2048 bins per core, 8 cores,
no communication. Accumulation over K=16384 happens in PSUM (fp32) via fp8e4m3
DoubleRow matmuls (0.5 PE cycles per output row, 256-deep contraction).

The vote matrix is binary, so the whole problem is transport-bound: dense fp8
V is 33.5 MB/core (~93us at the 360 B/ns DMA roofline), while the PE can
consume fp8 DR operands at ~1229 B/ns.  Mode "hyb" therefore splits V between
two producers that run concurrently with the PE:
  - 12 of the 64 DoubleRow chunk-pairs stream as dense fp8 via DMA;
  - 52 pairs ship bit-packed at 1 bit/elem: a uint16 lead group (8 pairs,
    one DVE tensor_scalar per pair extracts two bit-planes:
    (U & ((1<<s)|(1<<(s+8)))) << (6-s), 4x DVE mode, 594ns/pair) lets the
    DVE start as soon as the first 512KB lands; the remaining 44 pairs use
    uint32 words where ONE op extracts FOUR planes = two DR pairs
    ((U32 & 4-bit mask) shifted to 0x40 in every byte, 2x mode,
    562ns/pair). Either way the plane buffer bitcast to fp8 reads exactly
    2.0/0.0 in DoubleRow's byte-interleaved rhs layout.
DMA (~29us), DVE expansion (~30us) and PE (~27.4us + p-state warmup) are
all near-balanced; sim 39.9us vs 115.4us for the dense-fp8 baseline.
The final couple expands in two half-column ops into separate slot tiles:
psum_a closes while the second half is still expanding, PE's last wait is
4 matmuls, and the two epilogue scale ops start staggered (ACT takes the
early-closing half, DVE the late one).
x arrives in three column-slice DMAs matching three ACT relu slices,
ordered by PE consumption (v-pair cols, couple cols, dense cols), each in
its own tile pair (a shared tile would chain every relu behind the last
x DMA); the PE starts at ~5.5us and never idles mid-stream.
The accumulator is split into two 2-bank PSUM tiles so the epilogue's two
scale ops (DVE+ACT, bf16 out) run in parallel — Tile serializes cross-engine
readers of a shared tile — and the stores issue from SP (pre-issued) + ACT.

Numerics: V encodings (1.0 dense / 2.0 packed) are exact; per-chunk x scales
(16 dense / 8 packed, folded into the host-side fp8 cast of x) make one PSUM
accumulator hold 16*(x@V); the epilogue multiplies by 1/(128*16).  relu stays
on-device (fp8 rounding preserves sign, so relu(fp8(s*x)) == fp8(s*relu(x))).
Only the fp8 quantization of x and the bf16 output store are lossy:
rel_l2 ~ 4.1e-3 (gate 2e-2).

Older modes kept for reference: f32 | f16 | f8dr (hi/lo split) | f8s.
"""

import numpy as np

import concourse.bass as bass
import concourse.bacc as bacc
import concourse.mybir as mybir
import concourse.tile as tile
from concourse.bass_utils import run_bass_kernel_spmd

MODE = "hyb"  # one of: f32 | f16 | f8dr | f8s | hyb

NCORES = 8
B, C, ROWS, COLS, H, W = 2, 16, 128, 128, 128, 128
BC = B * C                      # 32 output rows
K = ROWS * COLS                 # 16384 contraction
NTOT = H * W                    # 16384 output bins
NPC = NTOT // NCORES            # 2048 bins per core
KC = K // 128                   # 128 k-chunks of 128
NT = 512                        # matmul free-dim tile
X_SCALE = {"f32": 1.0, "f16": 1.0, "f8dr": 16.0, "f8s": 16.0}
OUT_SCALE = {"f32": 1.0 / COLS, "f16": 1.0 / COLS,
             "f8dr": 1.0 / (COLS * 16.0), "f8s": 1.0 / (COLS * 16.0)}
VDT = {
    "f32": mybir.dt.float32,
    "f16": mybir.dt.float16,
    "f8dr": mybir.dt.float8e4,
    "f8s": mybir.dt.float8e4,
}
# k-chunks per DMA block: keep each dma_start at 2 MiB (1 MiB for f8s)
GROUP = {"f32": 2, "f16": 4, "f8dr": 8, "f8s": 4}
VBUFS = {"f32": 4, "f16": 4, "f8dr": 4, "f8s": 4}

_nc_cache: dict[str, bass.Bass] = {}

# ---- hybrid-mode constants ----------------------------------------------
# 64 DoubleRow chunk-pairs (128 k-chunks of 128 rows) from two producers,
# balanced so DMA_ENGINES and DVE finish together:
#   - dense pairs arrive as fp8 via DMA (360 B/ns shared bus)
#   - packed pairs arrive as 1 bit/elem uint16 words; ONE DVE tensor_scalar
#     per pair extracts two bit-planes at once:
#       (U & ((1<<s)|(1<<(s+8)))) << (6-s)   [s=7: >> 1]
#     leaving 0x40 in the lo byte (plane j=0) and 0x4000's hi byte (plane
#     j=1); the uint16 buffer bitcast to fp8 reads 2.0/0.0 in exactly the
#     byte-interleaved layout DoubleRow wants (j stride 1, n stride 2).
# Scale bookkeeping: dense chunks use x*16 (V=1.0), packed use x*8 (V=2.0),
# so one PSUM accumulator holds 16*(x@V) and OUT_SCALE=1/(128*16) for both.
HYB_DPAIRS = 12                  # dense chunk-pairs, one per DMA block
HYB_PPAIRS = 64 - HYB_DPAIRS     # packed pairs (8 per uint16 word group)
HYB_GROUPS = (HYB_PPAIRS + 7) // 8
HYB_DVE_NS = 594                 # est ns per uint16 packed pair on DVE
HYB_U32_NS = 1125                # est ns per uint32 couple (2 pairs)
HYB_XFER_D = 1490                # est ns per dense pair DMA
HYB_XFER_G = 1456                # est ns per group / x DMA
HYB_DVE_BIAS = 2400              # est DVE pipeline-head offset (ns)
HYB_WARMUP = 20                  # PE p-state warmup matmuls

def _build(mode: str) -> bass.Bass:
    if mode in _nc_cache:
        return _nc_cache[mode]
    if mode == "hyb":
        nc = _build_hyb()
        _nc_cache[mode] = nc
        return nc
    vdt = VDT[mode]
    g = GROUP[mode]
    nb = KC // g
    f32 = mybir.dt.float32

    nc = bacc.Bacc("TRN2", target_bir_lowering=False, debug=False,
                   num_devices=NCORES)
    xdt = vdt if mode == "f8s" else f32
    x_dram = nc.dram_tensor("x", (128, KC * BC), xdt, kind="ExternalInput")
    v_dram = nc.dram_tensor("v", (nb, 128, g * NPC + 16), vdt,
                            kind="ExternalInput")
    o_dram = nc.dram_tensor("out", (BC, NPC), mybir.dt.bfloat16,
                            kind="ExternalOutput")

    vbufs = VBUFS[mode]
    with tile.TileContext(nc) as tc:
        with tc.tile_pool(name="xp", bufs=1) as xp, \
             tc.tile_pool(name="vp", bufs=1) as vp, \
             tc.tile_pool(name="pp", bufs=1, space="PSUM") as pp, \
             tc.tile_pool(name="pt", bufs=1, space="PSUM") as pt, \
             tc.tile_pool(name="gs", bufs=nb) as gate_pool, \
             tc.tile_pool(name="op", bufs=1) as op:

            # --- x preparation: load, relu(+scale), cast/split ---
            x_raw = xp.tile([128, KC * BC], xdt)
            nc.scalar.dma_start(out=x_raw[:], in_=x_dram.ap())

            relu = mybir.ActivationFunctionType.Relu
            if mode == "f8s":
                # host sent fp8e4m3(16*x); relu on DVE keeps the ACT queue
                # free to issue the V-block DMA stream without stalls
                x_use = xp.tile([128, KC * BC], vdt)
                nc.vector.tensor_relu(x_use[:], x_raw[:])
                passes = [x_use]
            elif mode == "f32":
                x_use = xp.tile([128, KC * BC], f32)
                nc.scalar.activation(x_use[:], x_raw[:], relu)
                passes = [x_use]
            elif mode == "f16":
                x_use = xp.tile([128, KC * BC], mybir.dt.float16)
                nc.scalar.activation(x_use[:], x_raw[:], relu)
                passes = [x_use]
            else:  # f8dr: hi/lo split of relu(x)*16
                x_rel = xp.tile([128, KC * BC], f32)
                nc.scalar.activation(x_rel[:], x_raw[:], relu,
                                     scale=X_SCALE[mode])
                x_hi = xp.tile([128, KC * BC], vdt)
                nc.vector.tensor_copy(out=x_hi[:], in_=x_rel[:])
                x_hi32 = xp.tile([128, KC * BC], f32)
                nc.vector.tensor_copy(out=x_hi32[:], in_=x_hi[:])
                resid = xp.tile([128, KC * BC], f32)
                nc.vector.tensor_sub(resid[:], x_rel[:], x_hi32[:])
                x_lo = xp.tile([128, KC * BC], vdt)
                nc.vector.tensor_copy(out=x_lo[:], in_=resid[:])
                passes = [x_hi, x_lo]

            # two accumulators (2 PSUM banks each): the epilogue's two
            # scale ops then read disjoint tiles and truly run in parallel
            psum_a = pp.tile([BC, NPC // 2], f32, tag="pa", name="psum_a")
            psum_b = pp.tile([BC, NPC // 2], f32, tag="pb", name="psum_b")

            # Walrus allows only ONE sem-wait per DMA instruction, but a
            # v-block DMA into a reused pool slot needs two: WAR on the
            # stale tile's PE readers + WAW on the slot's previous DMA
            # (Tile doesn't collapse waits transitively). Fix:
            #  - every block ends with a tiny "token" matmul into a
            #    dedicated PSUM bank (last PE op touching the block's tile)
            #  - before reusing a slot, ACT copies that token from PSUM
            #    into the stale tile: this gate carries the single PE wait
            #    and its write WAW-orders it before the real DMA on ACT
            #  - the real DMA (also issued from ACT) then carries only the
            #    DMA-lane WAW wait: every instruction has <= 1 sem wait.
            vtiles: list = []
            tok = []
            vts = []
            for j in range(vbufs):
                tok_t = pt.tile([1, 16], f32, tag=f"tok{j}")
                tok.append(tok_t)
                vt_t = vp.tile([128, g * NPC + 16], vdt, tag=f"vt{j}")
                vts.append(vt_t)
            def gate(b):
                if b >= vbufs:
                    stale = vtiles[b - vbufs]
                    # absorb the stale slot's DMA-lane tick into ACT
                    # program order (1 wait: old DMA lane); fresh scratch
                    # slot every time so no WAW self-wait accumulates
                    pg_t = gate_pool.tile([1, 16], f32, tag="pg")
                    nc.scalar.copy(pg_t[:], stale[0:1, 16:32])
                    # carry the PE release (1 wait: PE >= token-mm), and
                    # WAW-order the real DMA behind us on ACT via the junk
                    # pad columns (PE never reads those)
                    nc.scalar.copy(stale[0:1, g * NPC:g * NPC + 16],
                                   tok[(b - vbufs) % vbufs][:])

            def token_mm(b, vt2d, lhs_src):
                nc.tensor.matmul(tok[b % vbufs][:], lhsT=lhs_src[:, 0:1],
                                 rhs=vt2d[:, 0:16], start=True, stop=True)

            # --- main loop: stream V blocks, accumulate matmuls ---
            if mode in ("f8dr", "f8s"):
                dr = mybir.MatmulPerfMode.DoubleRow
                gg_per_block = g // 2
                for b in range(nb):
                    gate(b)
                    vt2d = vts[b % vbufs]
                    vtiles.append(vt2d)
                    nc.scalar.dma_start(out=vt2d[:], in_=v_dram.ap()[b])
                    vt = vt2d[:, 0:g * NPC].rearrange(
                        "p (gg j n) -> p gg j n", gg=gg_per_block, j=2)
                    for gg in range(gg_per_block):
                        cc = b * gg_per_block + gg   # 0..63 double-chunks
                        first = cc == 0
                        last = cc == KC // 2 - 1
                        for n in range(NPC // NT):
                            rhs = vt[:, gg, :, n * NT:(n + 1) * NT]
                            for ip, xpass in enumerate(passes):
                                lhsT = xpass[:, 2 * cc * BC:(2 * cc + 2) * BC]
                                lhsT = lhsT.rearrange(
                                    "p (j m) -> p j m", j=2)
                                nc.tensor.matmul(
                                    psum[:, n * NT:(n + 1) * NT],
                                    lhsT=lhsT, rhs=rhs,
                                    start=(first and ip == 0),
                                    stop=(last and ip == len(passes) - 1),
                                    perf_mode=dr)
                    token_mm(b, vt2d, passes[0])
            else:
                for b in range(nb):
                    gate(b)
                    vt = vts[b % vbufs]
                    vtiles.append(vt)
                    nc.scalar.dma_start(out=vt[:], in_=v_dram.ap()[b])
                    for i in range(g):
                        c = b * g + i
                        lhsT = passes[0][:, c * BC:(c + 1) * BC]
                        for n in range(NPC // NT):
                            nc.tensor.matmul(
                                psum[:, n * NT:(n + 1) * NT],
                                lhsT=lhsT,
                                rhs=vt[:, i * NPC + n * NT:
                                       i * NPC + (n + 1) * NT],
                                start=(c == 0), stop=(c == KC - 1))
                    token_mm(b, vt, passes[0])

            # --- epilogue: flush the last blocks' DMA-lane ticks into ACT
            # so the kernel-tail Drain doesn't exceed its wait capacity ---
            for bb in range(max(0, nb - vbufs), nb):
                fl_t = gate_pool.tile([1, 16], f32, tag="pg")
                nc.scalar.copy(fl_t[:], vtiles[bb][0:1, 16:32])

            # --- epilogue: scale + store ---
            out_t = op.tile([BC, NPC], f32)
            nc.scalar.mul(out_t[:], psum[:], OUT_SCALE[mode])
            nc.scalar.dma_start(out=o_dram.ap(), in_=out_t[:])

    nc.finalize()
    _nc_cache[mode] = nc
    return nc


def _build_hyb() -> bass.Bass:
    f32 = mybir.dt.float32
    f8 = mybir.dt.float8e4
    u16 = mybir.dt.uint16
    alu = mybir.AluOpType
    dr = mybir.MatmulPerfMode.DoubleRow
    nb = HYB_DPAIRS                  # one dense pair per DMA block
    ebufs = 8

    nc = bacc.Bacc("TRN2", target_bir_lowering=False, debug=False,
                   num_devices=NCORES)
    x_dram = nc.dram_tensor("x", (128, KC * BC), f8, kind="ExternalInput")
    v_dram = nc.dram_tensor("v", (nb, 128, 2 * NPC), f8,
                            kind="ExternalInput")
    u32 = mybir.dt.uint32
    n16 = 8                          # pairs in the uint16 lead group
    ncpl = (HYB_PPAIRS - n16) // 2   # uint32 "couples" (2 pairs per DVE op)
    g32 = (ncpl + 7) // 8            # uint32 word groups
    u_dram = nc.dram_tensor("u", (128, NPC), u16, kind="ExternalInput")
    w_dram = nc.dram_tensor("w", (g32, 128, NPC), u32, kind="ExternalInput")
    o_dram = nc.dram_tensor("out", (BC, NPC), mybir.dt.bfloat16,
                            kind="ExternalOutput")

    # ---- static schedule: estimate producer completion times -------------
    # Every dense pair has its own SBUF tile (no slot reuse -> no gating),
    # so ALL input DMAs issue ungated from the otherwise idle SP queue,
    # groups interleaved early so DVE never starves.
    # x is DMA'd in three column slices matching the relu slices (v-pair
    # cols, couple cols, dense cols) so PE's first lhsT is ready early
    xsz = {"x1": 64 * 128 * n16, "x3": 64 * 128 * 2 * ncpl,
           "x2": 64 * 128 * nb}
    sp_issue = [("u16",), ("x1",), ("x3",), ("u32", 0), ("x2",), ("d", 0),
                ("d", 1), ("d", 2), ("u32", 1), ("d", 3), ("d", 4),
                ("u32", 2), ("d", 5)] + [("d", b) for b in range(6, nb)]
    sp_issue = [it for it in sp_issue
                if not (it[0] == "u32" and it[1] >= g32)
                and not (it[0] == "d" and it[1] >= nb)]
    t, d_ready, g32_ready, g16_ready = 2900.0, {}, {}, 0.0
    for it in sp_issue:
        if it[0] in xsz:
            t += xsz[it[0]] / 360.0
        elif it[0] == "u16":
            t += HYB_XFER_G
            g16_ready = t
        elif it[0] == "u32":
            t += 2 * HYB_XFER_G
            g32_ready[it[1]] = t
        else:
            t += HYB_XFER_D
            d_ready[it[1]] = t
    # bias: observed DVE pipeline head (first-op sem chains) — biasing the
    # packed-pair estimates late keeps PE from idling on them (idle resets
    # the PE p-state ramp, halving matmul throughput)
    tt = float(HYB_DVE_BIAS)
    dve_est, cpl_est = {}, {}
    for l in range(n16):
        tt = max(tt, g16_ready) + HYB_DVE_NS
        dve_est[l] = tt
    for c in range(ncpl):
        tt = max(tt, g32_ready[c // 8]) + HYB_U32_NS
        cpl_est[c] = tt
    # hold the last dense pairs back to interleave with the final couples:
    # couples outpace PE consumption by ~270ns each, dense pairs (already
    # resident) fill those waits so PE finishes right behind the DVE
    if nb >= 3 and ncpl >= 6:
        d_ready[nb - 3] = cpl_est[ncpl - 5] - 1
        d_ready[nb - 2] = cpl_est[ncpl - 3] - 1
        d_ready[nb - 1] = cpl_est[ncpl - 1] - 1
    items = ([("d", b, d_ready[b]) for b in range(nb)]
             + [("v", l, dve_est[l]) for l in range(n16)]
             + [("c", c, cpl_est[c]) for c in range(ncpl)])
    items.sort(key=lambda it: it[2])

    with tile.TileContext(nc) as tc:
        with tc.tile_pool(name="xp", bufs=1) as xp, \
             tc.tile_pool(name="vp", bufs=1) as vp, \
             tc.tile_pool(name="gp", bufs=1) as gp, \
             tc.tile_pool(name="ep", bufs=1) as ep, \
             tc.tile_pool(name="pp", bufs=1, space="PSUM") as pp, \
             tc.tile_pool(name="op", bufs=1) as op:

            gt16 = gp.tile([128, NPC], u16, tag="g16", name="gt16")
            gt32 = [gp.tile([128, NPC], u32, tag=f"w{t_}",
                            name=f"wt{t_}") for t_ in range(g32)]
            # per-slice x tiles: slicing one tile would chain every relu
            # behind the LAST x DMA (tile-granular dependency tracking)
            nx1, nx3 = 64 * n16, 64 * 2 * ncpl
            nx2 = 64 * nb
            xr1 = xp.tile([128, nx1], f8, tag="xr1", name="xr1")
            xr2 = xp.tile([128, nx2], f8, tag="xr2", name="xr2")
            xr3 = xp.tile([128, nx3], f8, tag="xr3", name="xr3")
            xu1 = xp.tile([128, nx1], f8, tag="xu1", name="xu1")
            xu2 = xp.tile([128, nx2], f8, tag="xu2", name="xu2")
            xu3 = xp.tile([128, nx3], f8, tag="xu3", name="xu3")
            # two accumulators (2 PSUM banks each): the epilogue's two
            # scale ops then read disjoint tiles and truly run in parallel
            psum_a = pp.tile([BC, NPC // 2], f32, tag="pa", name="psum_a")
            psum_b = pp.tile([BC, NPC // 2], f32, tag="pb", name="psum_b")
            vts = [vp.tile([128, 2 * NPC], f8, tag=f"vt{b}",
                           name=f"vt{b}") for b in range(nb)]
            etiles = [ep.tile([128, NPC], u16, tag=f"e{j}",
                              name=f"et{j}") for j in range(8)]
            qtiles = [ep.tile([128, NPC], u32, tag=f"q{j}",
                              name=f"qt{j}") for j in range(ebufs)]
            qha = ep.tile([128, NPC // 2], u32, tag="qha", name="qha")
            qhb = ep.tile([128, NPC // 2], u32, tag="qhb", name="qhb")

            # ---- SP queue: every input DMA, ungated ----------------------
            for it in sp_issue:
                if it[0] == "u16":
                    nc.sync.dma_start(out=gt16[:], in_=u_dram.ap())
                elif it[0] == "u32":
                    nc.sync.dma_start(out=gt32[it[1]][:],
                                      in_=w_dram.ap()[it[1]])
                elif it[0] == "x1":
                    nc.sync.dma_start(
                        out=xr1[:],
                        in_=x_dram.ap()[:, 64 * nb:64 * (nb + n16)])
                elif it[0] == "x2":
                    nc.sync.dma_start(out=xr2[:],
                                      in_=x_dram.ap()[:, 0:64 * nb])
                elif it[0] == "x3":
                    nc.sync.dma_start(
                        out=xr3[:], in_=x_dram.ap()[:, 64 * (nb + n16):])
                else:
                    nc.sync.dma_start(out=vts[it[1]][:],
                                      in_=v_dram.ap()[it[1]])

            # ---- PE p-state warmup: ~40 junk matmuls into a spare PSUM
            # bank keep the PE continuously busy from ~0.7us so the ramp
            # (low->mid->full over 3us) completes before real work arrives;
            # every real matmul then runs at the full 0.42ns/row rate ------
            junk = xp.tile([128, 1024], f8, name="junk")
            nc.scalar.memzero(junk[:])
            jpsum = pp.tile([BC, NT], f32, tag="warm", name="jpsum")
            jlhs = junk[:, 0:2 * BC].rearrange("p (j m) -> p j m", j=2)
            jrhs = junk[:].rearrange("p (j n) -> p j n", j=2)
            for w in range(HYB_WARMUP):
                nc.tensor.matmul(jpsum[:], lhsT=jlhs, rhs=jrhs,
                                 start=True, stop=True, perf_mode=dr)

            # ---- ACT: relu in consumption order (uint16 v-pair cols,
            # couple cols, dense cols — dense pairs are consumed latest) ---
            relu = mybir.ActivationFunctionType.Relu
            nc.scalar.activation(xu1[:], xr1[:], relu)
            nc.scalar.activation(xu3[:], xr3[:], relu)
            nc.scalar.activation(xu2[:], xr2[:], relu)

            def pair_matmuls(xt, li, rhs_jn, first, last):
                lhsT = xt[:, 64 * li:64 * (li + 1)].rearrange(
                    "p (j m) -> p j m", j=2)
                for n in range(NPC // NT):
                    ps = psum_a if n < 2 else psum_b
                    nc.tensor.matmul(
                        ps[:, (n % 2) * NT:(n % 2 + 1) * NT], lhsT=lhsT,
                        rhs=rhs_jn[:, :, n * NT:(n + 1) * NT],
                        start=first, stop=last, perf_mode=dr)

            # ---- matmuls (+ inline expansion) in estimated-ready order ---
            e16cnt, e32cnt = 0, 0
            last_idx = len(items) - 1
            for idx, it in enumerate(items):
                first, last = idx == 0, idx == last_idx
                if it[0] == "d":
                    b = it[1]
                    rhs = vts[b][:].rearrange("p (j n) -> p j n", j=2)
                    pair_matmuls(xu2, b, rhs, first, last)
                elif it[0] == "v":
                    l = it[1]
                    slot = etiles[e16cnt % 8]
                    e16cnt += 1
                    if l <= 6:
                        op1, s2 = alu.logical_shift_left, 6 - l
                    else:
                        op1, s2 = alu.logical_shift_right, 1
                    nc.vector.tensor_scalar(
                        out=slot[:], in0=gt16[:],
                        scalar1=(1 << l) | (1 << (l + 8)), scalar2=s2,
                        op0=alu.bitwise_and, op1=op1)
                    rhs = slot[:].bitcast(f8).rearrange(
                        "p (n j) -> p j n", j=2)
                    pair_matmuls(xu1, l, rhs, first, last)
                else:
                    c = it[1]
                    gi, s2 = c // 8, c % 8
                    mask = ((1 << s2) | (1 << (s2 + 8)) | (1 << (s2 + 16))
                            | (1 << (s2 + 24)))
                    if s2 <= 6:
                        op1, sh = alu.logical_shift_left, 6 - s2
                    else:
                        op1, sh = alu.logical_shift_right, 1
                    if not last:
                        slot = qtiles[e32cnt % ebufs]
                        e32cnt += 1
                        nc.vector.tensor_scalar(
                            out=slot[:], in0=gt32[gi][:], scalar1=mask,
                            scalar2=sh, op0=alu.bitwise_and, op1=op1)
                        pv = slot[:].bitcast(f8).rearrange(
                            "p (n f) -> p f n", f=4)
                        for h in (0, 1):
                            pair_matmuls(xu3, 2 * c + h,
                                         pv[:, 2 * h:2 * h + 2, :],
                                         first and h == 0, False)
                        continue
                    # final couple: two half-column ops so psum_a closes
                    # while the second half is still expanding, and PE's
                    # last wait is 4 matmuls instead of 8
                    hn2 = NPC // 2
                    lhs = [xu3[:, 64 * (2 * c + h):64 * (2 * c + h + 1)]
                           .rearrange("p (j m) -> p j m", j=2)
                           for h in (0, 1)]
                    for half, qt in ((0, qha), (1, qhb)):
                        nc.vector.tensor_scalar(
                            out=qt[:],
                            in0=gt32[gi][:, half * hn2:(half + 1) * hn2],
                            scalar1=mask, scalar2=sh,
                            op0=alu.bitwise_and, op1=op1)
                        pvh = qt[:].bitcast(f8).rearrange(
                            "p (n f) -> p f n", f=4)
                        ps = psum_a if half == 0 else psum_b
                        for h in (0, 1):
                            for n in (0, 1):
                                nc.tensor.matmul(
                                    ps[:, n * NT:(n + 1) * NT],
                                    lhsT=lhs[h],
                                    rhs=pvh[:, 2 * h:2 * h + 2,
                                            n * NT:(n + 1) * NT],
                                    start=False, stop=h == 1,
                                    perf_mode=dr)

            # ---- epilogue: scale each PSUM quarter as it closes (DVE and
            # ACT alternate); interleaved-quarter stores go out via SP and
            # the gpsimd SWDGE queue (shorter post-wait issue chain) -------
            # separate tiles per half: a shared tile serializes the two
            # muls on Tile's tile-granular dependency tracking
            bf16 = mybir.dt.bfloat16
            hn = NPC // 2
            out_a = op.tile([BC, hn], bf16, tag="oa", name="out_a")
            out_b = op.tile([BC, hn], bf16, tag="ob", name="out_b")
            sc = 1.0 / (COLS * 16.0)
            # psum_a closes first (half-split final couple) -> ACT scales
            # it while DVE finishes the last expansion, then DVE takes b
            nc.scalar.mul(out_a[:], psum_a[:], sc)
            nc.vector.tensor_scalar_mul(out_b[:], psum_b[:], sc)
            nc.sync.dma_start(out=o_dram.ap()[:, 0:hn], in_=out_a[:])
            nc.sync.dma_start(out=o_dram.ap()[:, hn:], in_=out_b[:])

    nc.finalize()
    return nc


def _prep_inputs(image: np.ndarray, vote_index: np.ndarray, mode: str):
    if mode == "hyb":
        return _prep_inputs_hyb(image, vote_index)
    np_vdt = mybir.dt.np(VDT[mode])
    g = GROUP[mode]
    nb = KC // g

    # x arranged (128, KC*BC): [p, c*32+m] = image_flat[m, c*128+p] * X_SCALE
    x2 = np.ascontiguousarray(image.reshape(BC, K), dtype=np.float32)
    x_arr = np.ascontiguousarray(
        x2.reshape(BC, KC, 128).transpose(2, 1, 0)).reshape(128, KC * BC)
    if mode == "f8s":
        # pre-relu fp8 cast of 16*x: fp8 rounding preserves sign, so
        # relu(fp8(16x)) == fp8(16*relu(x)) — relu itself stays on device
        x_arr = (x_arr * X_SCALE[mode]).astype(np_vdt)

    # v arranged per core: (nb, 128, g*NPC): [b, p, g'*NPC+j] =
    #   V[(b*g+g')*128 + p, core*NPC + j]
    v2 = vote_index.reshape(K, NTOT)
    if np_vdt != np.float32:
        v2 = v2.astype(np_vdt)  # binary 0/1 -> lossless
    # reshape [b, g', p, core, j] -> transpose to [core, b, p, g', j]
    v5 = v2.reshape(nb, g, 128, NCORES, NPC).transpose(3, 0, 2, 1, 4)
    in_maps = []
    for i in range(NCORES):
        vi = np.zeros((nb, 128, g * NPC + 16), dtype=np_vdt)
        vi[:, :, :g * NPC] = v5[i].reshape(nb, 128, g * NPC)
        in_maps.append({"x": x_arr, "v": vi})
    return in_maps


def _prep_inputs_hyb(image: np.ndarray, vote_index: np.ndarray):
    np_f8 = mybir.dt.np(mybir.dt.float8e4)
    nb = HYB_DPAIRS
    dchunks = 2 * HYB_DPAIRS

    # x arranged (128, KC*BC): [p, c*32+m] = image_flat[m, c*128+p] * s(c)
    # s = 16 for dense chunks (V encoded as 1.0), 8 for packed (V reads 2.0)
    x2 = np.ascontiguousarray(image.reshape(BC, K), dtype=np.float32)
    x_arr = np.ascontiguousarray(
        x2.reshape(BC, KC, 128).transpose(2, 1, 0)).reshape(128, KC * BC)
    scales = np.where(np.arange(KC) < dchunks, 16.0, 8.0).astype(np.float32)
    x_arr = (x_arr.reshape(128, KC, BC) * scales[None, :, None]
             ).reshape(128, KC * BC).astype(np_f8)

    v2 = vote_index.reshape(K, NTOT)
    # dense file: (nb, 128, 2*NPC+16); block b col = j*NPC + n, chunk 2b+j
    vd = v2[:dchunks * 128].reshape(nb, 2, 128, NCORES, NPC)
    vd = vd.transpose(3, 0, 2, 1, 4)  # (core, b, p, j, n)
    # packed files: a uint16 lead group (8 pairs: chunk dchunks+2l+j ->
    # bit l+8j) + uint32 groups of 8 "couples" each (chunk
    # dchunks+16+4c+k -> group c//8, bit (c%8)+8k)
    pchunks = K // 128 - dchunks
    n16 = 8
    ncpl = (pchunks // 2 - n16) // 2
    g32 = (ncpl + 7) // 8
    vp16 = v2[dchunks * 128:(dchunks + 2 * n16) * 128].reshape(
        2 * n16, 128, NCORES, NPC)
    u16_all = np.zeros((128, NCORES, NPC), dtype=np.uint16)
    for q in range(2 * n16):
        u16_all |= vp16[q].astype(np.uint16) << np.uint16(q // 2 + 8 * (q % 2))
    c32 = pchunks - 2 * n16
    vp32 = v2[(dchunks + 2 * n16) * 128:].reshape(c32, 128, NCORES, NPC)
    w_all = np.zeros((g32, 128, NCORES, NPC), dtype=np.uint32)
    for q in range(c32):
        c, k = q // 4, q % 4
        w_all[c // 8] |= vp32[q].astype(np.uint32) << np.uint32(c % 8 + 8 * k)

    in_maps = []
    for i in range(NCORES):
        vi = np.ascontiguousarray(
            vd[i].reshape(nb, 128, 2 * NPC)).astype(np_f8)
        in_maps.append({"x": x_arr, "v": vi,
                        "u": np.ascontiguousarray(u16_all[:, i, :]),
                        "w": np.ascontiguousarray(w_all[:, :, i, :])})
    return in_maps


def _run(image, vote_index, mode=None, **run_kwargs):
    mode = mode or MODE
    nc = _build(mode)
    in_maps = _prep_inputs(np.asarray(image), np.asarray(vote_index), mode)
    res = run_bass_kernel_spmd(nc, in_maps, core_ids=list(range(NCORES)),
                               **run_kwargs)
    out = np.concatenate([r["out"] for r in res.results], axis=1)
    return out.reshape(B, C, H, W).astype(np.float32), res


def kernel(image: np.ndarray, vote_index: np.ndarray) -> np.ndarray:
    out, _ = _run(image, vote_index)
    return out



# revision 55
# speedup vs baseline: 1.0035x; 1.0035x over previous
"""Trainium2 Bass kernel for nn_HT_56298431316042 (histogram_binning).

Computes  out = relu(image.reshape(32, 16384)) @ vote.reshape(16384, 16384) / 128
         -> reshape (2, 16, 128, 128)

Sharding: column-wise over the 16384 Hough bins -> 2048 bins per core, 8 cores,
no communication. Accumulation over K=16384 happens in PSUM (fp32) via fp8e4m3
DoubleRow matmuls (0.5 PE cycles per output row, 256-deep contraction).

The vote matrix is binary, so the whole problem is transport-bound: dense fp8
V is 33.5 MB/core (~93us at the 360 B/ns DMA roofline), while the PE can
consume fp8 DR operands at ~1229 B/ns.  Mode "hyb" therefore splits V between
two producers that run concurrently with the PE:
  - 12 of the 64 DoubleRow chunk-pairs stream as dense fp8 via DMA;
  - 52 pairs ship bit-packed at 1 bit/elem: a uint16 lead group (8 pairs,
    one DVE tensor_scalar per pair extracts two bit-planes:
    (U & ((1<<s)|(1<<(s+8)))) << (6-s), 4x DVE mode, 594ns/pair) lets the
    DVE start as soon as the first 512KB lands; the remaining 44 pairs use
    uint32 words where ONE op extracts FOUR planes = two DR pairs
    ((U32 & 4-bit mask) shifted to 0x40 in every byte, 2x mode,
    562ns/pair). Either way the plane buffer bitcast to fp8 reads exactly
    2.0/0.0 in DoubleRow's byte-interleaved rhs layout.
DMA (~29us), DVE expansion (~30us) and PE (~27.4us + p-state warmup) are
all near-balanced; sim 39.9us vs 115.4us for the dense-fp8 baseline.
The final couple expands in two half-column ops into separate slot tiles:
psum_a closes while the second half is still expanding, PE's last wait is
4 matmuls, and the two epilogue scale ops start staggered (ACT takes the
early-closing half, DVE the late one).
x arrives in three column-slice DMAs matching three ACT relu slices,
ordered by PE consumption (v-pair cols, couple cols, dense cols), each in
its own tile pair (a shared tile would chain every relu behind the last
x DMA); the PE starts at ~5.5us and never idles mid-stream.
The accumulator is split into two 2-bank PSUM tiles so the epilogue's two
scale ops (DVE+ACT, bf16 out) run in parallel — Tile serializes cross-engine
readers of a shared tile — and the stores issue from SP (pre-issued) + ACT.

Numerics: V encodings (1.0 dense / 2.0 packed) are exact; per-chunk x scales
(16 dense / 8 packed, folded into the host-side fp8 cast of x) make one PSUM
accumulator hold 16*(x@V); the epilogue multiplies by 1/(128*16).  relu stays
on-device (fp8 rounding preserves sign, so relu(fp8(s*x)) == fp8(s*relu(x))).
Only the fp8 quantization of x and the bf16 output store are lossy:
rel_l2 ~ 4.1e-3 (gate 2e-2).

Older modes kept for reference: f32 | f16 | f8dr (hi/lo split) | f8s.
"""

import numpy as np

import concourse.bass as bass
import concourse.bacc as bacc
import concourse.mybir as mybir
import concourse.tile as tile
from concourse.bass_utils import run_bass_kernel_spmd

MODE = "hyb"  # one of: f32 | f16 | f8dr | f8s | hyb

NCORES = 8
B, C, ROWS, COLS, H, W = 2, 16, 128, 128, 128, 128
BC = B * C                      # 32 output rows
K = ROWS * COLS                 # 16384 contraction
NTOT = H * W                    # 16384 output bins
NPC = NTOT // NCORES            # 2048 bins per core
KC = K // 128                   # 128 k-chunks of 128
NT = 512                        # matmul free-dim tile
X_SCALE = {"f32": 1.0, "f16": 1.0, "f8dr": 16.0, "f8s": 16.0}
OUT_SCALE = {"f32": 1.0 / COLS, "f16": 1.0 / COLS,
             "f8dr": 1.0 / (COLS * 16.0), "f8s": 1.0 / (COLS * 16.0)}
VDT = {
    "f32": mybir.dt.float32,
    "f16": mybir.dt.float16,
    "f8dr": mybir.dt.float8e4,
    "f8s": mybir.dt.float8e4,
}
# k-chunks per DMA block: keep each dma_start at 2 MiB (1 MiB for f8s)
GROUP = {"f32": 2, "f16": 4, "f8dr": 8, "f8s": 4}
VBUFS = {"f32": 4, "f16": 4, "f8dr": 4, "f8s": 4}

_nc_cache: dict[str, bass.Bass] = {}

# ---- hybrid-mode constants ----------------------------------------------
# 64 DoubleRow chunk-pairs (128 k-chunks of 128 rows) from two producers,
# balanced so DMA_ENGINES and DVE finish together:
#   - dense pairs arrive as fp8 via DMA (360 B/ns shared bus)
#   - packed pairs arrive as 1 bit/elem uint16 words; ONE DVE tensor_scalar
#     per pair extracts two bit-planes at once:
#       (U & ((1<<s)|(1<<(s+8)))) << (6-s)   [s=7: >> 1]
#     leaving 0x40 in the lo byte (plane j=0) and 0x4000's hi byte (plane
#     j=1); the uint16 buffer bitcast to fp8 reads 2.0/0.0 in exactly the
#     byte-interleaved layout DoubleRow wants (j stride 1, n stride 2).
# Scale bookkeeping: dense chunks use x*16 (V=1.0), packed use x*8 (V=2.0),
# so one PSUM accumulator holds 16*(x@V) and OUT_SCALE=1/(128*16) for both.
HYB_DPAIRS = 12                  # dense chunk-pairs, one per DMA block
HYB_PPAIRS = 64 - HYB_DPAIRS     # packed pairs (8 per uint16 word group)
HYB_GROUPS = (HYB_PPAIRS + 7) // 8
HYB_DVE_NS = 594                 # est ns per uint16 packed pair on DVE
HYB_U32_NS = 1125                # est ns per uint32 couple (2 pairs)
HYB_XFER_D = 1490                # est ns per dense pair DMA
HYB_XFER_G = 1456                # est ns per group / x DMA
HYB_DVE_BIAS = 2400              # est DVE pipeline-head offset (ns)
HYB_WARMUP = 20                  # PE p-state warmup matmuls

def _build(mode: str) -> bass.Bass:
    if mode in _nc_cache:
        return _nc_cache[mode]
    if mode == "hyb":
        nc = _build_hyb()
        _nc_cache[mode] = nc
        return nc
    vdt = VDT[mode]
    g = GROUP[mode]
    nb = KC // g
    f32 = mybir.dt.float32

    nc = bacc.Bacc("TRN2", target_bir_lowering=False, debug=False,
                   num_devices=NCORES)
    xdt = vdt if mode == "f8s" else f32
    x_dram = nc.dram_tensor("x", (128, KC * BC), xdt, kind="ExternalInput")
    v_dram = nc.dram_tensor("v", (nb, 128, g * NPC + 16), vdt,
                            kind="ExternalInput")
    o_dram = nc.dram_tensor("out", (BC, NPC), mybir.dt.bfloat16,
                            kind="ExternalOutput")

    vbufs = VBUFS[mode]
    with tile.TileContext(nc) as tc:
        with tc.tile_pool(name="xp", bufs=1) as xp, \
             tc.tile_pool(name="vp", bufs=1) as vp, \
             tc.tile_pool(name="pp", bufs=1, space="PSUM") as pp, \
             tc.tile_pool(name="pt", bufs=1, space="PSUM") as pt, \
             tc.tile_pool(name="gs", bufs=nb) as gate_pool, \
             tc.tile_pool(name="op", bufs=1) as op:

            # --- x preparation: load, relu(+scale), cast/split ---
            x_raw = xp.tile([128, KC * BC], xdt)
            nc.scalar.dma_start(out=x_raw[:], in_=x_dram.ap())

            relu = mybir.ActivationFunctionType.Relu
            if mode == "f8s":
                # host sent fp8e4m3(16*x); relu on DVE keeps the ACT queue
                # free to issue the V-block DMA stream without stalls
                x_use = xp.tile([128, KC * BC], vdt)
                nc.vector.tensor_relu(x_use[:], x_raw[:])
                passes = [x_use]
            elif mode == "f32":
                x_use = xp.tile([128, KC * BC], f32)
                nc.scalar.activation(x_use[:], x_raw[:], relu)
                passes = [x_use]
            elif mode == "f16":
                x_use = xp.tile([128, KC * BC], mybir.dt.float16)
                nc.scalar.activation(x_use[:], x_raw[:], relu)
                passes = [x_use]
            else:  # f8dr: hi/lo split of relu(x)*16
                x_rel = xp.tile([128, KC * BC], f32)
                nc.scalar.activation(x_rel[:], x_raw[:], relu,
                                     scale=X_SCALE[mode])
                x_hi = xp.tile([128, KC * BC], vdt)
                nc.vector.tensor_copy(out=x_hi[:], in_=x_rel[:])
                x_hi32 = xp.tile([128, KC * BC], f32)
                nc.vector.tensor_copy(out=x_hi32[:], in_=x_hi[:])
                resid = xp.tile([128, KC * BC], f32)
                nc.vector.tensor_sub(resid[:], x_rel[:], x_hi32[:])
                x_lo = xp.tile([128, KC * BC], vdt)
                nc.vector.tensor_copy(out=x_lo[:], in_=resid[:])
                passes = [x_hi, x_lo]

            # two accumulators (2 PSUM banks each): the epilogue's two
            # scale ops then read disjoint tiles and truly run in parallel
            psum_a = pp.tile([BC, NPC // 2], f32, tag="pa", name="psum_a")
            psum_b = pp.tile([BC, NPC // 2], f32, tag="pb", name="psum_b")

            # Walrus allows only ONE sem-wait per DMA instruction, but a
            # v-block DMA into a reused pool slot needs two: WAR on the
            # stale tile's PE readers + WAW on the slot's previous DMA
            # (Tile doesn't collapse waits transitively). Fix:
            #  - every block ends with a tiny "token" matmul into a
            #    dedicated PSUM bank (last PE op touching the block's tile)
            #  - before reusing a slot, ACT copies that token from PSUM
            #    into the stale tile: this gate carries the single PE wait
            #    and its write WAW-orders it before the real DMA on ACT
            #  - the real DMA (also issued from ACT) then carries only the
            #    DMA-lane WAW wait: every instruction has <= 1 sem wait.
            vtiles: list = []
            tok = []
            vts = []
            for j in range(vbufs):
                tok_t = pt.tile([1, 16], f32, tag=f"tok{j}")
                tok.append(tok_t)
                vt_t = vp.tile([128, g * NPC + 16], vdt, tag=f"vt{j}")
                vts.append(vt_t)
            def gate(b):
                if b >= vbufs:
                    stale = vtiles[b - vbufs]
                    # absorb the stale slot's DMA-lane tick into ACT
                    # program order (1 wait: old DMA lane); fresh scratch
                    # slot every time so no WAW self-wait accumulates
                    pg_t = gate_pool.tile([1, 16], f32, tag="pg")
                    nc.scalar.copy(pg_t[:], stale[0:1, 16:32])
                    # carry the PE release (1 wait: PE >= token-mm), and
                    # WAW-order the real DMA behind us on ACT via the junk
                    # pad columns (PE never reads those)
                    nc.scalar.copy(stale[0:1, g * NPC:g * NPC + 16],
                                   tok[(b - vbufs) % vbufs][:])

            def token_mm(b, vt2d, lhs_src):
                nc.tensor.matmul(tok[b % vbufs][:], lhsT=lhs_src[:, 0:1],
                                 rhs=vt2d[:, 0:16], start=True, stop=True)

            # --- main loop: stream V blocks, accumulate matmuls ---
            if mode in ("f8dr", "f8s"):
                dr = mybir.MatmulPerfMode.DoubleRow
                gg_per_block = g // 2
                for b in range(nb):
                    gate(b)
                    vt2d = vts[b % vbufs]
                    vtiles.append(vt2d)
                    nc.scalar.dma_start(out=vt2d[:], in_=v_dram.ap()[b])
                    vt = vt2d[:, 0:g * NPC].rearrange(
                        "p (gg j n) -> p gg j n", gg=gg_per_block, j=2)
                    for gg in range(gg_per_block):
                        cc = b * gg_per_block + gg   # 0..63 double-chunks
                        first = cc == 0
                        last = cc == KC // 2 - 1
                        for n in range(NPC // NT):
                            rhs = vt[:, gg, :, n * NT:(n + 1) * NT]
                            for ip, xpass in enumerate(passes):
                                lhsT = xpass[:, 2 * cc * BC:(2 * cc + 2) * BC]
                                lhsT = lhsT.rearrange(
                                    "p (j m) -> p j m", j=2)
                                nc.tensor.matmul(
                                    psum[:, n * NT:(n + 1) * NT],
                                    lhsT=lhsT, rhs=rhs,
                                    start=(first and ip == 0),
                                    stop=(last and ip == len(passes) - 1),
                                    perf_mode=dr)
                    token_mm(b, vt2d, passes[0])
            else:
                for b in range(nb):
                    gate(b)
                    vt = vts[b % vbufs]
                    vtiles.append(vt)
                    nc.scalar.dma_start(out=vt[:], in_=v_dram.ap()[b])
                    for i in range(g):
                        c = b * g + i
                        lhsT = passes[0][:, c * BC:(c + 1) * BC]
                        for n in range(NPC // NT):
                            nc.tensor.matmul(
                                psum[:, n * NT:(n + 1) * NT],
                                lhsT=lhsT,
                                rhs=vt[:, i * NPC + n * NT:
                                       i * NPC + (n + 1) * NT],
                                start=(c == 0), stop=(c == KC - 1))
                    token_mm(b, vt, passes[0])

            # --- epilogue: flush the last blocks' DMA-lane ticks into ACT
            # so the kernel-tail Drain doesn't exceed its wait capacity ---
            for bb in range(max(0, nb - vbufs), nb):
                fl_t = gate_pool.tile([1, 16], f32, tag="pg")
                nc.scalar.copy(fl_t[:], vtiles[bb][0:1, 16:32])

            # --- epilogue: scale + store ---
            out_t = op.tile([BC, NPC], f32)
            nc.scalar.mul(out_t[:], psum[:], OUT_SCALE[mode])
            nc.scalar.dma_start(out=o_dram.ap(), in_=out_t[:])

    nc.finalize()
    _nc_cache[mode] = nc
    return nc


def _build_hyb() -> bass.Bass:
    f32 = mybir.dt.float32
    f8 = mybir.dt.float8e4
    u16 = mybir.dt.uint16
    alu = mybir.AluOpType
    dr = mybir.MatmulPerfMode.DoubleRow
    nb = HYB_DPAIRS                  # one dense pair per DMA block
    ebufs = 8

    nc = bacc.Bacc("TRN2", target_bir_lowering=False, debug=False,
                   num_devices=NCORES)
    x_dram = nc.dram_tensor("x", (128, KC * BC), f8, kind="ExternalInput")
    v_dram = nc.dram_tensor("v", (nb, 128, 2 * NPC), f8,
                            kind="ExternalInput")
    u32 = mybir.dt.uint32
    n16 = 8                          # pairs in the uint16 lead group
    ncpl = (HYB_PPAIRS - n16) // 2   # uint32 "couples" (2 pairs per DVE op)
    g32 = (ncpl + 7) // 8            # uint32 word groups
    u_dram = nc.dram_tensor("u", (128, NPC), u16, kind="ExternalInput")
    w_dram = nc.dram_tensor("w", (g32, 128, NPC), u32, kind="ExternalInput")
    o_dram = nc.dram_tensor("out", (BC, NPC), mybir.dt.bfloat16,
                            kind="ExternalOutput")

    # ---- static schedule: estimate producer completion times -------------
    # Every dense pair has its own SBUF tile (no slot reuse -> no gating),
    # so ALL input DMAs issue ungated from the otherwise idle SP queue,
    # groups interleaved early so DVE never starves.
    # x is DMA'd in three column slices matching the relu slices (v-pair
    # cols, couple cols, dense cols) so PE's first lhsT is ready early
    xsz = {"x1": 64 * 128 * n16, "x3": 64 * 128 * 2 * ncpl,
           "x2": 64 * 128 * nb}
    sp_issue = [("u16a",), ("u16b",), ("x1",), ("x3",), ("u32", 0),
                ("x2",), ("d", 0), ("d", 1), ("d", 2), ("u32", 1),
                ("d", 3), ("d", 4), ("u32", 2), ("d", 5)] + [
                ("d", b) for b in range(6, nb)]
    sp_issue = [it for it in sp_issue
                if not (it[0] == "u32" and it[1] >= g32)
                and not (it[0] == "d" and it[1] >= nb)]
    t, d_ready, g32_ready, g16_ready = 2900.0, {}, {}, 0.0
    for it in sp_issue:
        if it[0] in xsz:
            t += xsz[it[0]] / 360.0
        elif it[0] == "u16a":
            t += HYB_XFER_G / 2
            g16a_ready = t
        elif it[0] == "u16b":
            t += HYB_XFER_G / 2
            g16_ready = t
        elif it[0] == "u32":
            t += 2 * HYB_XFER_G
            g32_ready[it[1]] = t
        else:
            t += HYB_XFER_D
            d_ready[it[1]] = t
    # bias: observed DVE pipeline head (first-op sem chains) — biasing the
    # packed-pair estimates late keeps PE from idling on them (idle resets
    # the PE p-state ramp, halving matmul throughput)
    tt = float(HYB_DVE_BIAS)
    va_est, vb_est, cpl_est = {}, {}, {}
    half_ns = HYB_DVE_NS // 2 + 30
    for l in range(n16):
        tt = max(tt, g16a_ready) + half_ns
        va_est[l] = tt
    for l in range(n16):
        tt = max(tt, g16_ready) + half_ns
        vb_est[l] = tt
    for c in range(ncpl):
        tt = max(tt, g32_ready[c // 8]) + HYB_U32_NS
        cpl_est[c] = tt
    # hold the last dense pairs back to interleave with the final couples:
    # couples outpace PE consumption by ~270ns each, dense pairs (already
    # resident) fill those waits so PE finishes right behind the DVE
    if nb >= 3 and ncpl >= 6:
        d_ready[nb - 3] = cpl_est[ncpl - 5] - 1
        d_ready[nb - 2] = cpl_est[ncpl - 3] - 1
        d_ready[nb - 1] = cpl_est[ncpl - 1] - 1
    items = ([("d", b, d_ready[b]) for b in range(nb)]
             + [("va", l, va_est[l]) for l in range(n16)]
             + [("vb", l, vb_est[l]) for l in range(n16)]
             + [("c", c, cpl_est[c]) for c in range(ncpl)])
    items.sort(key=lambda it: it[2])

    with tile.TileContext(nc) as tc:
        with tc.tile_pool(name="xp", bufs=1) as xp, \
             tc.tile_pool(name="vp", bufs=1) as vp, \
             tc.tile_pool(name="gp", bufs=1) as gp, \
             tc.tile_pool(name="ep", bufs=1) as ep, \
             tc.tile_pool(name="pp", bufs=1, space="PSUM") as pp, \
             tc.tile_pool(name="op", bufs=1) as op:

            gt16a = gp.tile([128, NPC // 2], u16, tag="g16a", name="gt16a")
            gt16b = gp.tile([128, NPC // 2], u16, tag="g16b", name="gt16b")
            gt32 = [gp.tile([128, NPC], u32, tag=f"w{t_}",
                            name=f"wt{t_}") for t_ in range(g32)]
            # per-slice x tiles: slicing one tile would chain every relu
            # behind the LAST x DMA (tile-granular dependency tracking)
            nx1, nx3 = 64 * n16, 64 * 2 * ncpl
            nx2 = 64 * nb
            xr1 = xp.tile([128, nx1], f8, tag="xr1", name="xr1")
            xr2 = xp.tile([128, nx2], f8, tag="xr2", name="xr2")
            xr3 = xp.tile([128, nx3], f8, tag="xr3", name="xr3")
            xu1 = xp.tile([128, nx1], f8, tag="xu1", name="xu1")
            xu2 = xp.tile([128, nx2], f8, tag="xu2", name="xu2")
            xu3 = xp.tile([128, nx3], f8, tag="xu3", name="xu3")
            # two accumulators (2 PSUM banks each): the epilogue's two
            # scale ops then read disjoint tiles and truly run in parallel
            psum_a = pp.tile([BC, NPC // 2], f32, tag="pa", name="psum_a")
            psum_b = pp.tile([BC, NPC // 2], f32, tag="pb", name="psum_b")
            vts = [vp.tile([128, 2 * NPC], f8, tag=f"vt{b}",
                           name=f"vt{b}") for b in range(nb)]
            etilesA = [ep.tile([128, NPC // 2], u16, tag=f"ea{j}",
                               name=f"ea{j}") for j in range(8)]
            etilesB = [ep.tile([128, NPC // 2], u16, tag=f"eb{j}",
                               name=f"eb{j}") for j in range(8)]
            qtiles = [ep.tile([128, NPC], u32, tag=f"q{j}",
                              name=f"qt{j}") for j in range(ebufs)]
            qha = ep.tile([128, NPC // 2], u32, tag="qha", name="qha")
            qb1 = ep.tile([128, NPC // 4], u32, tag="qb1", name="qb1")
            qb2 = ep.tile([128, NPC // 4], u32, tag="qb2", name="qb2")

            # ---- SP queue: every input DMA, ungated ----------------------
            for it in sp_issue:
                if it[0] == "u16a":
                    nc.sync.dma_start(out=gt16a[:],
                                      in_=u_dram.ap()[:, 0:NPC // 2])
                elif it[0] == "u16b":
                    nc.sync.dma_start(out=gt16b[:],
                                      in_=u_dram.ap()[:, NPC // 2:])
                elif it[0] == "u32":
                    nc.sync.dma_start(out=gt32[it[1]][:],
                                      in_=w_dram.ap()[it[1]])
                elif it[0] == "x1":
                    nc.sync.dma_start(
                        out=xr1[:],
                        in_=x_dram.ap()[:, 64 * nb:64 * (nb + n16)])
                elif it[0] == "x2":
                    nc.sync.dma_start(out=xr2[:],
                                      in_=x_dram.ap()[:, 0:64 * nb])
                elif it[0] == "x3":
                    nc.sync.dma_start(
                        out=xr3[:], in_=x_dram.ap()[:, 64 * (nb + n16):])
                else:
                    nc.sync.dma_start(out=vts[it[1]][:],
                                      in_=v_dram.ap()[it[1]])

            # ---- PE p-state warmup: ~40 junk matmuls into a spare PSUM
            # bank keep the PE continuously busy from ~0.7us so the ramp
            # (low->mid->full over 3us) completes before real work arrives;
            # every real matmul then runs at the full 0.42ns/row rate ------
            junk = xp.tile([128, 1024], f8, name="junk")
            nc.scalar.memzero(junk[:])
            jpsum = pp.tile([BC, NT], f32, tag="warm", name="jpsum")
            jlhs = junk[:, 0:2 * BC].rearrange("p (j m) -> p j m", j=2)
            jrhs = junk[:].rearrange("p (j n) -> p j n", j=2)
            for w in range(HYB_WARMUP):
                nc.tensor.matmul(jpsum[:], lhsT=jlhs, rhs=jrhs,
                                 start=True, stop=True, perf_mode=dr)

            # ---- ACT: relu in consumption order (uint16 v-pair cols,
            # couple cols, dense cols — dense pairs are consumed latest) ---
            relu = mybir.ActivationFunctionType.Relu
            nc.scalar.activation(xu1[:], xr1[:], relu)
            nc.scalar.activation(xu3[:], xr3[:], relu)
            nc.scalar.activation(xu2[:], xr2[:], relu)

            started = set()

            def mm(ps, pofs, lhsT, rhs, stop):
                key = (id(ps), pofs)
                nc.tensor.matmul(ps[:, pofs:pofs + NT], lhsT=lhsT, rhs=rhs,
                                 start=key not in started, stop=stop,
                                 perf_mode=dr)
                started.add(key)

            def pair_matmuls(xt, li, rhs_jn, first, last):
                lhsT = xt[:, 64 * li:64 * (li + 1)].rearrange(
                    "p (j m) -> p j m", j=2)
                for n in range(NPC // NT):
                    ps = psum_a if n < 2 else psum_b
                    mm(ps, (n % 2) * NT, lhsT,
                       rhs_jn[:, :, n * NT:(n + 1) * NT], last)

            # ---- matmuls (+ inline expansion) in estimated-ready order ---
            e16cnt, e32cnt = 0, 0
            last_idx = len(items) - 1
            for idx, it in enumerate(items):
                first, last = idx == 0, idx == last_idx
                if it[0] == "d":
                    b = it[1]
                    rhs = vts[b][:].rearrange("p (j n) -> p j n", j=2)
                    pair_matmuls(xu2, b, rhs, first, last)
                elif it[0] in ("va", "vb"):
                    l = it[1]
                    is_a = it[0] == "va"
                    slot = (etilesA if is_a else etilesB)[l]
                    gt = gt16a if is_a else gt16b
                    if l <= 6:
                        op1, s2 = alu.logical_shift_left, 6 - l
                    else:
                        op1, s2 = alu.logical_shift_right, 1
                    nc.vector.tensor_scalar(
                        out=slot[:], in0=gt[:],
                        scalar1=(1 << l) | (1 << (l + 8)), scalar2=s2,
                        op0=alu.bitwise_and, op1=op1)
                    rhs = slot[:].bitcast(f8).rearrange(
                        "p (n j) -> p j n", j=2)
                    lhsT = xu1[:, 64 * l:64 * (l + 1)].rearrange(
                        "p (j m) -> p j m", j=2)
                    ps = psum_a if is_a else psum_b
                    for n in (0, 1):
                        mm(ps, n * NT, lhsT,
                           rhs[:, :, n * NT:(n + 1) * NT], False)
                else:
                    c = it[1]
                    gi, s2 = c // 8, c % 8
                    mask = ((1 << s2) | (1 << (s2 + 8)) | (1 << (s2 + 16))
                            | (1 << (s2 + 24)))
                    if s2 <= 6:
                        op1, sh = alu.logical_shift_left, 6 - s2
                    else:
                        op1, sh = alu.logical_shift_right, 1
                    if not last:
                        slot = qtiles[e32cnt % ebufs]
                        e32cnt += 1
                        nc.vector.tensor_scalar(
                            out=slot[:], in0=gt32[gi][:], scalar1=mask,
                            scalar2=sh, op0=alu.bitwise_and, op1=op1)
                        pv = slot[:].bitcast(f8).rearrange(
                            "p (n f) -> p f n", f=4)
                        for h in (0, 1):
                            pair_matmuls(xu3, 2 * c + h,
                                         pv[:, 2 * h:2 * h + 2, :],
                                         False, False)
                        continue
                    # final couple: expanded in shrinking column pieces
                    # (1024 + 512 + 512) into separate tiles, so each PSUM
                    # region closes while the next piece is still
                    # expanding and the epilogue starts earliest
                    lhs = [xu3[:, 64 * (2 * c + h):64 * (2 * c + h + 1)]
                           .rearrange("p (j m) -> p j m", j=2)
                           for h in (0, 1)]
                    pieces = ((qha, 0, 1024, psum_a, 0),
                              (qb1, 1024, 512, psum_b, 0),
                              (qb2, 1536, 512, psum_b, 512))
                    for qt, cofs, w, ps, pofs in pieces:
                        nc.vector.tensor_scalar(
                            out=qt[:], in0=gt32[gi][:, cofs:cofs + w],
                            scalar1=mask, scalar2=sh,
                            op0=alu.bitwise_and, op1=op1)
                        pvh = qt[:].bitcast(f8).rearrange(
                            "p (n f) -> p f n", f=4)
                        for h in (0, 1):
                            for n in range(w // NT):
                                mm(ps, pofs + n * NT, lhs[h],
                                   pvh[:, 2 * h:2 * h + 2,
                                       n * NT:(n + 1) * NT], h == 1)

            # ---- epilogue: scale each PSUM quarter as it closes (DVE and
            # ACT alternate); interleaved-quarter stores go out via SP and
            # the gpsimd SWDGE queue (shorter post-wait issue chain) -------
            # separate tiles per half: a shared tile serializes the two
            # muls on Tile's tile-granular dependency tracking
            bf16 = mybir.dt.bfloat16
            hn = NPC // 2
            out_a = op.tile([BC, hn], bf16, tag="oa", name="out_a")
            out_b = op.tile([BC, hn], bf16, tag="ob", name="out_b")
            sc = 1.0 / (COLS * 16.0)
            # psum_a closes first (half-split final couple) -> ACT scales
            # it while DVE finishes the last expansion, then DVE takes b
            nc.scalar.mul(out_a[:], psum_a[:], sc)
            nc.vector.tensor_scalar_mul(out_b[:], psum_b[:], sc)
            nc.sync.dma_start(out=o_dram.ap()[:, 0:hn], in_=out_a[:])
            nc.sync.dma_start(out=o_dram.ap()[:, hn:], in_=out_b[:])

    nc.finalize()
    return nc


def _prep_inputs(image: np.ndarray, vote_index: np.ndarray, mode: str):
    if mode == "hyb":
        return _prep_inputs_hyb(image, vote_index)
    np_vdt = mybir.dt.np(VDT[mode])
    g = GROUP[mode]
    nb = KC // g

    # x arranged (128, KC*BC): [p, c*32+m] = image_flat[m, c*128+p] * X_SCALE
    x2 = np.ascontiguousarray(image.reshape(BC, K), dtype=np.float32)
    x_arr = np.ascontiguousarray(
        x2.reshape(BC, KC, 128).transpose(2, 1, 0)).reshape(128, KC * BC)
    if mode == "f8s":
        # pre-relu fp8 cast of 16*x: fp8 rounding preserves sign, so
        # relu(fp8(16x)) == fp8(16*relu(x)) — relu itself stays on device
        x_arr = (x_arr * X_SCALE[mode]).astype(np_vdt)

    # v arranged per core: (nb, 128, g*NPC): [b, p, g'*NPC+j] =
    #   V[(b*g+g')*128 + p, core*NPC + j]
    v2 = vote_index.reshape(K, NTOT)
    if np_vdt != np.float32:
        v2 = v2.astype(np_vdt)  # binary 0/1 -> lossless
    # reshape [b, g', p, core, j] -> transpose to [core, b, p, g', j]
    v5 = v2.reshape(nb, g, 128, NCORES, NPC).transpose(3, 0, 2, 1, 4)
    in_maps = []
    for i in range(NCORES):
        vi = np.zeros((nb, 128, g * NPC + 16), dtype=np_vdt)
        vi[:, :, :g * NPC] = v5[i].reshape(nb, 128, g * NPC)
        in_maps.append({"x": x_arr, "v": vi})
    return in_maps


def _prep_inputs_hyb(image: np.ndarray, vote_index: np.ndarray):
    np_f8 = mybir.dt.np(mybir.dt.float8e4)
    nb = HYB_DPAIRS
    dchunks = 2 * HYB_DPAIRS

    # x arranged (128, KC*BC): [p, c*32+m] = image_flat[m, c*128+p] * s(c)
    # s = 16 for dense chunks (V encoded as 1.0), 8 for packed (V reads 2.0)
    x2 = np.ascontiguousarray(image.reshape(BC, K), dtype=np.float32)
    x_arr = np.ascontiguousarray(
        x2.reshape(BC, KC, 128).transpose(2, 1, 0)).reshape(128, KC * BC)
    scales = np.where(np.arange(KC) < dchunks, 16.0, 8.0).astype(np.float32)
    x_arr = (x_arr.reshape(128, KC, BC) * scales[None, :, None]
             ).reshape(128, KC * BC).astype(np_f8)

    v2 = vote_index.reshape(K, NTOT)
    # dense file: (nb, 128, 2*NPC+16); block b col = j*NPC + n, chunk 2b+j
    vd = v2[:dchunks * 128].reshape(nb, 2, 128, NCORES, NPC)
    vd = vd.transpose(3, 0, 2, 1, 4)  # (core, b, p, j, n)
    # packed files: a uint16 lead group (8 pairs: chunk dchunks+2l+j ->
    # bit l+8j) + uint32 groups of 8 "couples" each (chunk
    # dchunks+16+4c+k -> group c//8, bit (c%8)+8k)
    pchunks = K // 128 - dchunks
    n16 = 8
    ncpl = (pchunks // 2 - n16) // 2
    g32 = (ncpl + 7) // 8
    vp16 = v2[dchunks * 128:(dchunks + 2 * n16) * 128].reshape(
        2 * n16, 128, NCORES, NPC)
    u16_all = np.zeros((128, NCORES, NPC), dtype=np.uint16)
    for q in range(2 * n16):
        u16_all |= vp16[q].astype(np.uint16) << np.uint16(q // 2 + 8 * (q % 2))
    c32 = pchunks - 2 * n16
    vp32 = v2[(dchunks + 2 * n16) * 128:].reshape(c32, 128, NCORES, NPC)
    w_all = np.zeros((g32, 128, NCORES, NPC), dtype=np.uint32)
    for q in range(c32):
        c, k = q // 4, q % 4
        w_all[c // 8] |= vp32[q].astype(np.uint32) << np.uint32(c % 8 + 8 * k)

    in_maps = []
    for i in range(NCORES):
        vi = np.ascontiguousarray(
            vd[i].reshape(nb, 128, 2 * NPC)).astype(np_f8)
        in_maps.append({"x": x_arr, "v": vi,
                        "u": np.ascontiguousarray(u16_all[:, i, :]),
                        "w": np.ascontiguousarray(w_all[:, :, i, :])})
    return in_maps


def _run(image, vote_index, mode=None, **run_kwargs):
    mode = mode or MODE
    nc = _build(mode)
    in_maps = _prep_inputs(np.asarray(image), np.asarray(vote_index), mode)
    res = run_bass_kernel_spmd(nc, in_maps, core_ids=list(range(NCORES)),
                               **run_kwargs)
    out = np.concatenate([r["out"] for r in res.results], axis=1)
    return out.reshape(B, C, H, W).astype(np.float32), res


def kernel(image: np.ndarray, vote_index: np.ndarray) -> np.ndarray:
    out, _ = _run(image, vote_index)
    return out

